# Initial kernel scaffold
#
"""Trainium2 Bass kernel for nn_DecoderBlock (T=S=1024, B=4, E=1024, H=16,
DH=64, DFF=4096) on 8 NeuronCores.

Sharding: core = b*2 + th  (b in 0..3 batches, th in {0,1} query halves).
Each core computes the full decoder block for its 512 query rows of its
batch; no collectives. The SPMD program is identical on all cores; all
per-core differences (which batch, which query half, attention masks) are
carried by the input data prepared host-side.

Mask exploitation is data-driven: the host inspects the actual masks,
derives the number of live 128-wide key blocks (kv extents), and ships 0/1
multiplicative mask tiles only where a block is partially masked. With the
reference masks (causal + last-quarter key padding) this skips 25% of K/V
work and the fully-masked score blocks beyond the kv extent.

Matmuls run in bf16 with fp32 PSUM accumulation; softmax/normalization/
layernorm statistics run in fp32.
"""
import sys

sys.path.insert(0, "/opt/trn_rl_repo")

import numpy as np
import ml_dtypes

import concourse.bass as bass
from concourse import bacc
import concourse.mybir as mybir
import concourse.tile as tile
from concourse.bass_utils import run_bass_kernel_spmd
from concourse.masks import make_identity

F32 = mybir.dt.float32
BF16 = mybir.dt.bfloat16
AF = mybir.ActivationFunctionType
ALU = mybir.AluOpType
BF16NP = ml_dtypes.bfloat16

T = 1024
B = 4
E = 1024
H = 16
DH = 64
DFF = 4096
P = 128
TLOC = T // 2          # query rows per core
NTB = TLOC // P        # 4 query-row blocks per core
NEC = E // P           # 8 feature chunks
NFC = DFF // P         # 32 ffn chunks

_PROGRAM_CACHE = {}


def _attention(nc, po, *, yqT, kvT, kbs, wq, wk, wv, wp, bp_sb,
               mask_T=None, resid, out_pre, tag):
    """One multi-head attention + projection + residual.

    yqT:  SBUF tile [128, NEC, TLOC] bf16 — query-side activations, T-layout
    kvT:  SBUF tile [128, NEC, kbs*128] bf16 — key/value side, T-layout
    wq/wk/wv/wp: DRAM [E, E] bf16 (head-major cols; wq pre-scaled 1/sqrt(dh))
    mask_T: SBUF tile [128, kbs, TLOC] bf16 0/1 keep-mask or None
    resid/out_pre: [128, NTB, E] f32 R-layout tiles
    """
    SKV = kbs * P
    nsh = (SKV + 511) // 512

    # ---- Q^T and K^T per head-pair ----
    qT = po["cols"].tile([P, NEC, TLOC], BF16, tag="colsBF", name=f"{tag}_qT")
    kT = po["attn"].tile([P, NEC, SKV], BF16, tag="kvT", name=f"{tag}_kT")
    for p in range(8):
        wq_t = po["w"].tile([P, NEC, P], BF16, tag="w_qk", bufs=3, name=f"{tag}_wq{p}")
        nc.sync.dma_start(
            wq_t[:],
            wq[:, p * P:(p + 1) * P].rearrange("(eo pp) c -> pp eo c", pp=P))
        ps = po["psA"].tile([P, 512], F32, tag="sps", name=f"{tag}_psq{p}")
        for ec in range(NEC):
            nc.tensor.matmul(ps[:, :TLOC], wq_t[:, ec, :], yqT[:, ec, :],
                             start=(ec == 0), stop=(ec == NEC - 1))
        nc.vector.tensor_copy(qT[:, p, :], ps[:, :TLOC])

        wk_t = po["w"].tile([P, NEC, P], BF16, tag="w_qk", bufs=3, name=f"{tag}_wk{p}")
        nc.sync.dma_start(
            wk_t[:],
            wk[:, p * P:(p + 1) * P].rearrange("(eo pp) c -> pp eo c", pp=P))
        for sh in range(nsh):
            w = min(512, SKV - sh * 512)
            ps = po["psA"].tile([P, 512], F32, tag="sps",
                                name=f"{tag}_psk{p}_{sh}")
            for ec in range(NEC):
                nc.tensor.matmul(ps[:, :w], wk_t[:, ec, :],
                                 kvT[:, ec, sh * 512: sh * 512 + w],
                                 start=(ec == 0), stop=(ec == NEC - 1))
            nc.vector.tensor_copy(kT[:, p, sh * 512: sh * 512 + w], ps[:, :w])

    # ---- V in R-layout [s-chunk partitions, head cols] ----
    vR = po["attn"].tile([P, kbs, E], BF16, tag="vR", bufs=1,
                         name=f"{tag}_vR")
    for eh in range(2):
        wv_t = po["w"].tile([P, NEC, 512], BF16, tag="w_v", bufs=3,
                            name=f"{tag}_wv{eh}")
        nc.sync.dma_start(
            wv_t[:], wv[:, eh * 512:(eh + 1) * 512].rearrange(
                "(eo pp) c -> pp eo c", pp=P))
        for sc in range(kbs):
            ps = po["psA"].tile([P, 512], F32, tag="sps",
                                name=f"{tag}_psv{eh}_{sc}")
            for ec in range(NEC):
                nc.tensor.matmul(ps[:], kvT[:, ec, sc * P:(sc + 1) * P],
                                 wv_t[:, ec, :],
                                 start=(ec == 0), stop=(ec == NEC - 1))
            nc.vector.tensor_copy(vR[:, sc, eh * 512:(eh + 1) * 512], ps[:])

    # ---- attention per group of 4 heads (2 pairs) ----
    # Emission order targets HW concurrency: score matmuls for a head pair
    # are adjacent (row groups 0/64 run concurrently), attnV matmuls for the
    # 4 heads are adjacent (col groups), denominator matmuls are contiguous
    # (shared ones lhsT loaded once, 4-way col-group concurrency).
    oT = po["cols"].tile([P, NEC, TLOC], BF16, tag="colsBF", name=f"{tag}_oT")
    for g in range(4):
        dps = po["psB"].tile([P, 512], F32, tag="dps", name=f"{tag}_dps{g}")
        zts = {}
        pTs = {}
        for p in (2 * g, 2 * g + 1):
            for half in range(2):
                h = 2 * p + half
                pTs[h] = po["attn"].tile([P, kbs, TLOC], BF16, tag="pT",
                                         bufs=5, name=f"{tag}_pT{h}")
            for kb in range(kbs):
                sps2 = []
                for half in range(2):
                    hb = 64 * half
                    sps = po["psA"].tile([P, 512], F32, tag="sps",
                                         name=f"{tag}_sps{p}_{kb}_{half}")
                    nc.tensor.matmul(
                        sps[:, :TLOC],
                        kT[hb:hb + 64, p, kb * P:(kb + 1) * P],
                        qT[hb:hb + 64, p, :],
                        start=True, stop=True, tile_position=(hb, 0))
                    sps2.append(sps)
                for half in range(2):
                    h = 2 * p + half
                    nc.scalar.activation(pTs[h][:, kb, :],
                                         sps2[half][:, :TLOC], AF.Exp)
                    if mask_T is not None:
                        nc.vector.tensor_tensor(
                            pTs[h][:, kb, :], pTs[h][:, kb, :],
                            mask_T[:, kb, :], ALU.mult)
        for p in (2 * g, 2 * g + 1):
            zts[p] = po["psB"].tile([P, 512], F32, tag="zt",
                                    name=f"{tag}_zt{p}")
        for kb in range(kbs):
            for p in (2 * g, 2 * g + 1):
                for half in range(2):
                    h = 2 * p + half
                    hb = 64 * half
                    nc.tensor.matmul(
                        zts[p][hb:hb + 64, :TLOC],
                        vR[:, kb, h * 64:(h + 1) * 64],
                        pTs[h][:, kb, :],
                        start=(kb == 0), stop=(kb == kbs - 1),
                        tile_position=(0, hb))
        for kb in range(kbs):
            for gh in range(4):
                h = 4 * g + gh
                nc.tensor.matmul(
                    dps[32 * gh:32 * gh + 1, :TLOC],
                    po["ones_bf"][:, 0:1], pTs[h][:, kb, :],
                    start=(kb == 0), stop=(kb == kbs - 1),
                    tile_position=(0, 32 * gh))
        # denominators -> reciprocals
        dsb = po["scr"].tile([P, 512], F32, tag="dsb", name=f"{tag}_dsb{g}")
        nc.vector.memset(dsb[:], 1.0)
        for gh in range(4):
            nc.scalar.copy(dsb[32 * gh:32 * gh + 1, :],
                           dps[32 * gh:32 * gh + 1, :])
        rsb = po["scr"].tile([P, 512], F32, tag="rsb", name=f"{tag}_rsb{g}")
        nc.vector.reciprocal(out=rsb[:], in_=dsb[:])
        # broadcast recips and normalize
        for p in (2 * g, 2 * g + 1):
            bcp = po["psA"].tile([P, 512], F32, tag="sps",
                                 name=f"{tag}_bcp{p}")
            for half in range(2):
                h = 2 * p + half
                gh = h % 4
                hb = 64 * half
                nc.tensor.matmul(
                    bcp[hb:hb + 64, :TLOC],
                    po["ones_f32"][32 * gh:32 * gh + 1, 0:64],
                    rsb[32 * gh:32 * gh + 1, :TLOC],
                    start=True, stop=True,
                    tile_position=(32 * gh, hb))
            bcs = po["scr"].tile([P, 512], F32, tag="bcs", name=f"{tag}_bc{p}")
            nc.scalar.copy(bcs[:, :TLOC], bcp[:, :TLOC])
            nc.vector.tensor_tensor(oT[:, p, :], zts[p][:, :TLOC],
                                    bcs[:, :TLOC], ALU.mult)

    # ---- projection + bias + residual ----
    for tb in range(NTB):
        for eh in range(2):
            ps = po["psA"].tile([P, 512], F32, tag="sps",
                                name=f"{tag}_pspr{tb}_{eh}")
            for p in range(8):
                wp_t = po["w"].tile([P, 512], BF16, tag="w_p", bufs=4,
                                    name=f"{tag}_wp{tb}_{eh}_{p}")
                nc.sync.dma_start(
                    wp_t[:],
                    wp[p * P:(p + 1) * P, eh * 512:(eh + 1) * 512])
                nc.tensor.matmul(ps[:], oT[:, p, tb * P:(tb + 1) * P],
                                 wp_t[:], start=(p == 0), stop=False)
            nc.tensor.matmul(ps[:], po["ones_bf"][0:1, 0:P],
                             bp_sb[0:1, eh * 512:(eh + 1) * 512],
                             start=False, stop=True)
            nc.vector.tensor_tensor(
                out_pre[:, tb, eh * 512:(eh + 1) * 512], ps[:],
                resid[:, tb, eh * 512:(eh + 1) * 512], ALU.add)


def _ln_and_transpose(nc, po, *, src, outR, dst_T=None, gb=None, tag=""):
    """Per-row layernorm of [128, NTB, E] f32 + optional bf16 transpose to
    T-layout [128, NEC, TLOC]."""
    for tb in range(NTB):
        stats = po["scr"].tile([P, 2, 6], F32, tag="ln_st",
                               name=f"{tag}_st{tb}")
        nc.vector.bn_stats(stats[:, 0, :], src[:, tb, 0:512])
        nc.vector.bn_stats(stats[:, 1, :], src[:, tb, 512:1024])
        mv = po["scr"].tile([P, 2], F32, tag="ln_mv", name=f"{tag}_mv{tb}")
        nc.vector.bn_aggr(mv[:], stats[:])
        nmean = po["scr"].tile([P, 1], F32, tag="ln_nm", name=f"{tag}_nm{tb}")
        nc.vector.tensor_scalar_mul(nmean[:], mv[:, 0:1], -1.0)
        rstd = po["scr"].tile([P, 1], F32, tag="ln_rs", name=f"{tag}_rs{tb}")
        nc.scalar.activation(rstd[:], mv[:, 1:2], AF.Sqrt,
                             bias=po["eps"][:])
        nc.vector.reciprocal(out=rstd[:], in_=rstd[:])
        nc.vector.tensor_scalar(outR[:, tb, :], src[:, tb, :],
                                nmean[:], rstd[:], ALU.add, ALU.mult)
        if gb is not None:
            g_bc, b_bc = gb
            nc.vector.tensor_tensor(outR[:, tb, :], outR[:, tb, :],
                                    g_bc[:], ALU.mult)
            nc.vector.tensor_tensor(outR[:, tb, :], outR[:, tb, :],
                                    b_bc[:], ALU.add)
        if dst_T is not None:
            ybf = po["scr"].tile([P, E], BF16, tag="ybf", name=f"{tag}_yb{tb}")
            nc.vector.tensor_copy(ybf[:], outR[:, tb, :])
            for eg in range(2):
                pt = po["psA"].tile([P, 4, P], BF16, tag="sps",
                                    name=f"{tag}_tr{tb}_{eg}")
                for j in range(4):
                    ec = eg * 4 + j
                    nc.tensor.transpose(pt[:, j, :],
                                        ybf[:, ec * P:(ec + 1) * P],
                                        po["ident"][:])
                for j in range(4):
                    ec = eg * 4 + j
                    nc.scalar.copy(dst_T[:, ec, tb * P:(tb + 1) * P],
                                   pt[:, j, :])


def _broadcast_row(nc, po, src_row, width, tag):
    """Broadcast [1, width] f32 SBUF row (base 0) -> [128, width] f32."""
    out = po["persist"].tile([P, E], F32, tag=tag, name=tag)
    for c in range(0, width, 512):
        w = min(512, width - c)
        ps = po["psA"].tile([P, 512], F32, tag="sps", name=f"{tag}_bc{c}")
        nc.tensor.matmul(ps[0:P, :w], po["ones_f32"][0:1, 0:P],
                         src_row[0:1, c:c + w], start=True, stop=True)
        nc.scalar.copy(out[:, c:c + w], ps[:, :w])
    return out


def build_program(kbs_s, kbs_c, use_mask_s, use_mask_c, use_gb):
    nc = bacc.Bacc("TRN2", target_bir_lowering=False, debug=False,
                   num_devices=8)
    SKV_S = kbs_s * P
    SKV_C = kbs_c * P

    def di(name, shape, dt=BF16):
        return nc.dram_tensor(name, shape, dt, kind="ExternalInput")

    xTq = di("xTq", [E, TLOC])
    xTkv = di("xTkv", [E, SKV_S])
    xres = di("xres", [TLOC, E], F32)
    yencT = di("yencT", [E, SKV_C])
    wq1 = di("wq1", [E, E]); wk1 = di("wk1", [E, E]); wv1 = di("wv1", [E, E])
    wp1 = di("wp1", [E, E]); bp1 = di("bp1", [1, E])
    wq2 = di("wq2", [E, E]); wk2 = di("wk2", [E, E]); wv2 = di("wv2", [E, E])
    wp2 = di("wp2", [E, E]); bp2 = di("bp2", [1, E])
    w1 = di("w1", [E, DFF]); b1c = di("b1c", [P, NFC], F32)
    w2 = di("w2", [DFF, E]); b2 = di("b2", [1, E])
    if use_mask_s:
        mask_s = di("mask_s", [SKV_S, TLOC])
    if use_mask_c:
        mask_c = di("mask_c", [SKV_C, TLOC])
    if use_gb:
        lngb = di("lngb", [1, 6 * E], F32)
    out = nc.dram_tensor("out", [TLOC, E], F32, kind="ExternalOutput")

    with tile.TileContext(nc) as tc:
        with (
            tc.tile_pool(name="persist", bufs=1) as persist,
            tc.tile_pool(name="rows", bufs=2) as rows,
            tc.tile_pool(name="cols", bufs=3) as cols,
            tc.tile_pool(name="wpool", bufs=2) as wpool,
            tc.tile_pool(name="scr", bufs=2) as scr,
            tc.tile_pool(name="psA", bufs=4, space="PSUM") as psA,
        ):
            po = dict(persist=persist, rows=rows, cols=cols, w=wpool,
                      scr=scr, psA=psA)

            ones_bf = persist.tile([P, P], BF16)
            nc.vector.memset(ones_bf[:], 1.0)
            ones_f32 = persist.tile([P, 64], F32)
            nc.vector.memset(ones_f32[:], 1.0)
            ident = persist.tile([P, P], BF16)
            make_identity(nc, ident[:])
            eps_tile = persist.tile([P, 1], F32)
            nc.vector.memset(eps_tile[:], 1e-5)
            po.update(ones_bf=ones_bf, ones_f32=ones_f32, ident=ident,
                      eps=eps_tile)

            bp1_sb = persist.tile([1, E], BF16, tag="bp1", name="bp1s")
            nc.sync.dma_start(bp1_sb[:], bp1[:])
            bp2_sb = persist.tile([1, E], BF16, tag="bp2", name="bp2s")
            nc.sync.dma_start(bp2_sb[:], bp2[:])
            b2_sb = persist.tile([1, E], BF16, tag="b2", name="b2s")
            nc.sync.dma_start(b2_sb[:], b2[:])
            b1_sb = persist.tile([P, NFC], F32, tag="b1c", name="b1s")
            nc.sync.dma_start(b1_sb[:], b1c[:])

            gbs = [None, None, None]
            if use_gb:
                gbrow = persist.tile([1, 6 * E], F32, tag="lngb", name="gbr")
                nc.sync.dma_start(gbrow[:], lngb[:])
                for i in range(3):
                    g_bc = _broadcast_row(
                        nc, po, gbrow[:, 2 * i * E:(2 * i + 1) * E], E,
                        f"g_bc{i}")
                    b_bc = _broadcast_row(
                        nc, po, gbrow[:, (2 * i + 1) * E:(2 * i + 2) * E], E,
                        f"b_bc{i}")
                    gbs[i] = (g_bc, b_bc)

            xres_sb = rows.tile([P, NTB, E], F32, tag="rowsF32", name="xresS")
            for tb in range(NTB):
                nc.sync.dma_start(xres_sb[:, tb, :],
                                  xres[tb * P:(tb + 1) * P, :])
            y1pre = rows.tile([P, NTB, E], F32, tag="rowsF32", name="y1pre")
            y1R = rows.tile([P, NTB, E], F32, tag="rowsF32", name="y1R")
            y1T = cols.tile([P, NEC, TLOC], BF16, tag="colsBF", name="y1T")
            y2pre = rows.tile([P, NTB, E], F32, tag="rowsF32", name="y2pre")
            y2R = rows.tile([P, NTB, E], F32, tag="rowsF32", name="y2R")
            y2T = cols.tile([P, NEC, TLOC], BF16, tag="colsBF", name="y2T")

            with (
                tc.tile_pool(name="attn_sb", bufs=2) as attn_sb,
                tc.tile_pool(name="psB", bufs=2, space="PSUM") as psB,
            ):
                po["attn"] = attn_sb
                po["psB"] = psB

                mask_s_sb = None
                if use_mask_s:
                    mask_s_sb = attn_sb.tile([P, kbs_s, TLOC], BF16,
                                             tag="mask_s", bufs=1,
                                             name="mask_sS")
                    nc.sync.dma_start(
                        mask_s_sb[:],
                        mask_s.rearrange("(kb p) t -> p kb t", p=P))
                mask_c_sb = None
                if use_mask_c:
                    mask_c_sb = attn_sb.tile([P, kbs_c, TLOC], BF16,
                                             tag="mask_c", bufs=1,
                                             name="mask_cS")
                    nc.sync.dma_start(
                        mask_c_sb[:],
                        mask_c.rearrange("(kb p) t -> p kb t", p=P))

                xTq_sb = cols.tile([P, NEC, TLOC], BF16, tag="colsBF",
                                   name="xTqS")
                for ec in range(NEC):
                    nc.sync.dma_start(
                        xTq_sb[:, ec, :], xTq[ec * P:(ec + 1) * P, :])
                xTkv_sb = attn_sb.tile([P, NEC, SKV_S], BF16, tag="kvT",
                                       name="xTkvS")
                for ec in range(NEC):
                    nc.sync.dma_start(
                        xTkv_sb[:, ec, :], xTkv[ec * P:(ec + 1) * P, :])

                _attention(nc, po, yqT=xTq_sb, kvT=xTkv_sb, kbs=kbs_s,
                           wq=wq1, wk=wk1, wv=wv1, wp=wp1, bp_sb=bp1_sb,
                           mask_T=mask_s_sb, resid=xres_sb, out_pre=y1pre,
                           tag="sa")
                _ln_and_transpose(nc, po, src=y1pre, outR=y1R, dst_T=y1T,
                                  gb=gbs[0], tag="ln1")

                yencT_sb = attn_sb.tile([P, NEC, SKV_C], BF16, tag="kvT",
                                        name="yencTS")
                for ec in range(NEC):
                    nc.sync.dma_start(
                        yencT_sb[:, ec, :], yencT[ec * P:(ec + 1) * P, :])
                _attention(nc, po, yqT=y1T, kvT=yencT_sb, kbs=kbs_c,
                           wq=wq2, wk=wk2, wv=wv2, wp=wp2, bp_sb=bp2_sb,
                           mask_T=mask_c_sb, resid=y1R, out_pre=y2pre,
                           tag="ca")
                _ln_and_transpose(nc, po, src=y2pre, outR=y2R, dst_T=y2T,
                                  gb=gbs[1], tag="ln2")

            with (
                tc.tile_pool(name="ffn_sb", bufs=1) as ffn_sb,
                tc.tile_pool(name="psC", bufs=4, space="PSUM") as psC,
            ):
                hT = ffn_sb.tile([P, NFC, TLOC], BF16, tag="hT", name="hT")
                for fc in range(NFC):
                    w1_t = wpool.tile([P, NEC, P], BF16, tag="w_f1", bufs=3,
                                      name=f"w1_{fc}")
                    nc.sync.dma_start(
                        w1_t[:],
                        w1[:, fc * P:(fc + 1) * P].rearrange(
                            "(eo pp) c -> pp eo c", pp=P))
                    ps = psA.tile([P, 512], F32, tag="sps", name=f"psf1_{fc}")
                    for ec in range(NEC):
                        nc.tensor.matmul(ps[:, :TLOC], w1_t[:, ec, :],
                                         y2T[:, ec, :],
                                         start=(ec == 0),
                                         stop=(ec == NEC - 1))
                    nc.scalar.activation(hT[:, fc, :], ps[:, :TLOC], AF.Relu,
                                         bias=b1_sb[:, fc:fc + 1])

                y3pre = rows.tile([P, NTB, E], F32, tag="rowsF32",
                                  name="y3pre")
                for eh in range(2):
                    pss = [psC.tile([P, 512], F32, tag="ps_f2",
                                    name=f"psf2_{eh}_{tb}")
                           for tb in range(NTB)]
                    for fc in range(NFC):
                        w2_t = wpool.tile([P, 512], BF16, tag="w_f2", bufs=4,
                                          name=f"w2_{eh}_{fc}")
                        nc.sync.dma_start(
                            w2_t[:], w2[fc * P:(fc + 1) * P,
                                        eh * 512:(eh + 1) * 512])
                        for tb in range(NTB):
                            nc.tensor.matmul(
                                pss[tb][:], hT[:, fc, tb * P:(tb + 1) * P],
                                w2_t[:], start=(fc == 0), stop=False)
                    for tb in range(NTB):
                        nc.tensor.matmul(pss[tb][:], ones_bf[0:1, 0:P],
                                         b2_sb[0:1, eh * 512:(eh + 1) * 512],
                                         start=False, stop=True)
                        nc.vector.tensor_tensor(
                            y3pre[:, tb, eh * 512:(eh + 1) * 512],
                            pss[tb][:],
                            y2R[:, tb, eh * 512:(eh + 1) * 512], ALU.add)

                outR = rows.tile([P, NTB, E], F32, tag="rowsF32", name="outR")
                _ln_and_transpose(nc, po, src=y3pre, outR=outR, gb=gbs[2],
                                  tag="ln3")
                nc.sync.dma_start(out.rearrange("(tb p) e -> p tb e", p=P),
                                  outR[:])

    nc.compile()
    return nc


def _prep_inputs(inputs):
    """Host-side prep: returns (program_key, 8 in_maps, host_ln3)."""
    tgt = np.asarray(inputs["tgt"], np.float32)
    yenc = np.asarray(inputs["Y_enc_out"], np.float32)
    tgt_mask = np.asarray(inputs["tgt_mask"], np.float32)
    spm = np.asarray(inputs["src_padding_mask"])
    tpm = np.asarray(inputs["tgt_padding_mask"])

    causal = np.isneginf(tgt_mask) | np.isnan(tgt_mask)   # [Tq, Sk]
    masked_s = causal[None, :, :] | tpm[:, None, :]       # [B, Tq, Sk]
    masked_c = np.zeros((B, T, T), bool) | spm[:, None, :]

    live_s = ~masked_s.all(axis=(0, 1))
    live_c = ~masked_c.all(axis=(0, 1))
    kbs_s = max(1, -(-int(np.max(np.nonzero(live_s)[0], initial=0) + 1) // P))
    kbs_c = max(1, -(-int(np.max(np.nonzero(live_c)[0], initial=0) + 1) // P))

    keep_s = (~masked_s[:, :, :kbs_s * P]).astype(np.float32)
    keep_c = (~masked_c[:, :, :kbs_c * P]).astype(np.float32)
    use_mask_s = not np.all(keep_s == 1.0)
    use_mask_c = not np.all(keep_c == 1.0)

    g1 = np.asarray(inputs["ln1_g"], np.float32)
    b1g = np.asarray(inputs["ln1_b"], np.float32)
    g2 = np.asarray(inputs["ln2_g"], np.float32)
    b2g = np.asarray(inputs["ln2_b"], np.float32)
    g3 = np.asarray(inputs["ln3_g"], np.float32)
    b3g = np.asarray(inputs["ln3_b"], np.float32)
    use_gb = not (np.all(g1 == 1) and np.all(g2 == 1) and np.all(b1g == 0)
                  and np.all(b2g == 0))
    host_ln3 = None
    if not (np.all(g3 == 1) and np.all(b3g == 0)):
        host_ln3 = (g3, b3g)

    def heads_cols(w):  # [H, E, DH] -> [E, H*DH]
        return np.ascontiguousarray(
            np.asarray(w, np.float32).transpose(1, 0, 2).reshape(E, E))

    scale = 1.0 / np.sqrt(np.float32(DH))
    wq1 = (heads_cols(inputs["Wq1"]) * scale).astype(BF16NP)
    wk1 = heads_cols(inputs["Wk1"]).astype(BF16NP)
    wv1 = heads_cols(inputs["Wv1"]).astype(BF16NP)
    wq2 = (heads_cols(inputs["Wq2"]) * scale).astype(BF16NP)
    wk2 = heads_cols(inputs["Wk2"]).astype(BF16NP)
    wv2 = heads_cols(inputs["Wv2"]).astype(BF16NP)
    wp1 = np.asarray(inputs["Wp1"], np.float32).astype(BF16NP)
    wp2 = np.asarray(inputs["Wp2"], np.float32).astype(BF16NP)
    w1 = np.asarray(inputs["W1"], np.float32).astype(BF16NP)
    w2 = np.asarray(inputs["W2"], np.float32).astype(BF16NP)
    bp1 = np.asarray(inputs["bp1"], np.float32).reshape(1, E).astype(BF16NP)
    bp2 = np.asarray(inputs["bp2"], np.float32).reshape(1, E).astype(BF16NP)
    b2v = np.asarray(inputs["b2"], np.float32).reshape(1, E).astype(BF16NP)
    b1c = np.ascontiguousarray(
        np.asarray(inputs["b1"], np.float32).reshape(NFC, P).T)
    lngb = np.concatenate([g1, b1g, g2, b2g, g3, b3g]).reshape(1, 6 * E)

    in_maps = []
    for core in range(8):
        b = core // 2
        th = core % 2
        t0 = th * TLOC
        xb = tgt[:, b, :]
        xT = np.ascontiguousarray(xb.T)
        m = {
            "xTq": np.ascontiguousarray(xT[:, t0:t0 + TLOC]).astype(BF16NP),
            "xTkv": np.ascontiguousarray(xT[:, :kbs_s * P]).astype(BF16NP),
            "xres": np.ascontiguousarray(xb[t0:t0 + TLOC, :]),
            "yencT": np.ascontiguousarray(
                yenc[:kbs_c * P, b, :].T).astype(BF16NP),
            "wq1": wq1, "wk1": wk1, "wv1": wv1, "wp1": wp1, "bp1": bp1,
            "wq2": wq2, "wk2": wk2, "wv2": wv2, "wp2": wp2, "bp2": bp2,
            "w1": w1, "b1c": b1c, "w2": w2, "b2": b2v,
        }
        if use_mask_s:
            m["mask_s"] = np.ascontiguousarray(
                keep_s[b, t0:t0 + TLOC, :].T).astype(BF16NP)
        if use_mask_c:
            m["mask_c"] = np.ascontiguousarray(
                keep_c[b, t0:t0 + TLOC, :].T).astype(BF16NP)
        if use_gb:
            m["lngb"] = lngb
        in_maps.append(m)

    key = (kbs_s, kbs_c, use_mask_s, use_mask_c, use_gb)
    return key, in_maps, host_ln3


def kernel(**inputs) -> np.ndarray:
    key, in_maps, host_ln3 = _prep_inputs(inputs)
    if key not in _PROGRAM_CACHE:
        _PROGRAM_CACHE[key] = build_program(*key)
    nc = _PROGRAM_CACHE[key]
    res = run_bass_kernel_spmd(nc, in_maps, core_ids=list(range(8)))
    out = np.empty((T, B, E), np.float32)
    for core in range(8):
        b = core // 2
        th = core % 2
        out[th * TLOC:(th + 1) * TLOC, b, :] = res.results[core]["out"]
    if host_ln3 is not None:
        g3, b3g = host_ln3
        out = out * g3 + b3g
    return out



# revision 17
# speedup vs baseline: 1.8634x; 1.8634x over previous
"""Trainium2 Bass kernel for nn_DecoderBlock (T=S=1024, B=4, E=1024, H=16,
DH=64, DFF=4096) on 8 NeuronCores.

Sharding: core = b*2 + th (b in 0..3 batches, th in {0,1}). Core th owns the
four 128-row query blocks {th, th+2, th+4, th+6} (cyclic), which balances the
causal-mask live area across the two cores of a batch and lets the program
skip fully-masked (query-block, key-block) pairs exactly.

The host classifies every (query-block j, key-block kb) pair across all 8
cores into SKIP / FULL / MASKED; the SPMD program is built for that class
pattern (cached by pattern). MASKED blocks multiply the exp'd scores by a
per-core 0/1 mask tile.

Perf structure:
 - softmax group pipeline: emission order AV(g-1), denom(g-1), scores(g),
   bcp(g-1), norm(g-1) keeps recip-dependent matmuls away from the PE queue
   head; reciprocal via the fast custom-DVE op.
 - single ACT table set (natural_log_exp): layernorm rstd = exp(-0.5*ln(v+eps))
 - weights pre-chunked host-side for contiguous per-partition DMA, streamed
   on the gpsimd queue; Wv/Wp preloaded whole.
 - cross-attn K/V matmuls overlap LN1; per-tb LN2/LN3 overlap proj/FFN tails.
"""
import sys

sys.path.insert(0, "/opt/trn_rl_repo")

import numpy as np
import ml_dtypes

import concourse.bass as bass
from concourse import bacc
import concourse.mybir as mybir
import concourse.tile as tile
from concourse.bass_utils import run_bass_kernel_spmd
from concourse.masks import make_identity

F32 = mybir.dt.float32
BF16 = mybir.dt.bfloat16
AF = mybir.ActivationFunctionType
ALU = mybir.AluOpType
BF16NP = ml_dtypes.bfloat16

# Pin every activation function this kernel uses to the one table set that
# contains them all (natural_log_exp_and_others), so the whole kernel needs a
# single ACT_TABLE_LOAD instead of ping-ponging between exp/ln sets on every
# layernorm. Set names and their order (= act_func_set_id) are preserved; we
# only remove our functions from the other sets so the load-insertion pass
# cannot pick them.
_KERNEL_ACT_FUNCS = {AF.Exp, AF.Ln, AF.Relu, AF.Copy, AF.Identity}
_COMBINED_SET = "natural_log_exp_and_others"
_orig_get_act_tables = bacc.get_activation_tables


def _pinned_act_tables(arch):
    t = _orig_get_act_tables(arch)
    if _COMBINED_SET not in t or not (_KERNEL_ACT_FUNCS <= t[_COMBINED_SET]):
        return t
    return {name: (fns if name == _COMBINED_SET else fns - _KERNEL_ACT_FUNCS)
            for name, fns in t.items()}


bacc.get_activation_tables = _pinned_act_tables

T = 1024
B = 4
E = 1024
H = 16
DH = 64
DFF = 4096
P = 128
TLOC = T // 2          # query rows per core
NTB = TLOC // P        # 4 query-row blocks per core
NEC = E // P           # 8 feature chunks
NFC = DFF // P         # 32 ffn chunks

_PROGRAM_CACHE = {}


def _attn_k(nc, po, *, kvT, plan, wkc, kT, tag):
    """K^T for one attention. kvT: SBUF [128, NEC, SKV] bf16.
    wkc: DRAM [8, 128, NEC, 128] chunked."""
    kbs = max(e[0] for e in plan) + 1
    SKV = kbs * P
    nsh = (SKV + 511) // 512
    for p in range(8):
        wk_t = po["w"].tile([P, NEC, P], BF16, tag="w_qk", bufs=5,
                            name=f"{tag}_wk{p}")
        nc.sync.dma_start(wk_t[:], wkc[p])
        pss = [po["ps"].tile([P, 512], F32, tag="sps", name=f"{tag}_psk{p}_{sh}")
               for sh in range(nsh)]
        for ec in range(NEC):
            for sh in range(nsh):
                w = min(512, SKV - sh * 512)
                nc.tensor.matmul(pss[sh][:, :w], wk_t[:, ec, :],
                                 kvT[:, ec, sh * 512: sh * 512 + w],
                                 start=(ec == 0), stop=(ec == NEC - 1))
        for sh in range(nsh):
            w = min(512, SKV - sh * 512)
            nc.vector.tensor_copy(kT[:, p, sh * 512: sh * 512 + w],
                                  pss[sh][:, :w])


def _attn_v_units(nc, po, *, kvT, plan, wvc, vR, tag):
    """Returns (preload_fn, [unit closures]) for the V-phase: one closure per
    key block sc. Emit the closures anywhere PE filler work is needed."""
    kbs = max(e[0] for e in plan) + 1
    state = {}

    def preload():
        wv_sb = po["wbig"].tile([P, NEC, E], BF16, tag="wbig", bufs=2,
                                name=f"{tag}_wvsb")
        nc.scalar.dma_start(wv_sb[:], wvc[:])
        state["wv"] = wv_sb

    def unit(sc):
        def emit():
            wv_sb = state["wv"]
            psv = [po["ps"].tile([P, 512], F32, tag="sps",
                                 name=f"{tag}_psv{sc}_{eh}")
                   for eh in range(2)]
            for ec in range(NEC):
                for eh in range(2):
                    nc.tensor.matmul(psv[eh][:],
                                     kvT[:, ec, sc * P:(sc + 1) * P],
                                     wv_sb[:, ec, eh * 512:(eh + 1) * 512],
                                     start=(ec == 0), stop=(ec == NEC - 1))
            for eh in range(2):
                nc.vector.tensor_copy(vR[:, sc, eh * 512:(eh + 1) * 512],
                                      psv[eh][:])
        return emit

    return preload, [unit(sc) for sc in range(kbs)]


def _attn_main(nc, po, *, yqT, plan, mask_sb, wqc, wpc, kT, vR, bias_sb,
               resid, out_pre, tag, tb_cb=None, fillers=(), proj_by_group=False):
    """Q projection, attention groups (pipelined softmax), output projection.

    plan: tuple of (kb, n0, masked_j_tuple) for live key blocks, ascending kb.
    mask_sb: SBUF [128, nmask, 128] bf16 keep masks or None.
    tb_cb: optional callback(tb) emitted right after proj+residual of tb.
    """
    ones_bf = po["ones_bf"]
    kb_first = plan[0][0]
    kb_last = plan[-1][0]
    mask_idx = {}
    mi = 0
    for kb, n0, mjs in plan:
        for j in mjs:
            mask_idx[(kb, j)] = mi
            mi += 1

    # ---- Q^T ----
    qT = po["cols"].tile([P, NEC, TLOC], BF16, tag="colsBF", name=f"{tag}_qT")
    for p in range(8):
        wq_t = po["w"].tile([P, NEC, P], BF16, tag="w_qk", bufs=5,
                            name=f"{tag}_wq{p}")
        nc.sync.dma_start(wq_t[:], wqc[p])
        ps = po["ps"].tile([P, 512], F32, tag="sps", name=f"{tag}_psq{p}")
        for ec in range(NEC):
            nc.tensor.matmul(ps[:, :TLOC], wq_t[:, ec, :], yqT[:, ec, :],
                             start=(ec == 0), stop=(ec == NEC - 1))
        nc.vector.tensor_copy(qT[:, p, :], ps[:, :TLOC])

    # preload wp for the projection (single big DMA, off critical path)
    wp_sb = po["wbig"].tile([P, NEC, E], BF16, tag="wbig", bufs=2,
                            name=f"{tag}_wpsb")
    nc.scalar.dma_start(wp_sb[:], wpc[:])

    oT = po["cols"].tile([P, NEC, TLOC], BF16, tag="colsBF", name=f"{tag}_oT")
    pTs = {}
    zts = {}
    dps = {}

    def emit_scores(g):
        for p in (2 * g, 2 * g + 1):
            for half in range(2):
                h = 2 * p + half
                pTs[h] = po["attn"].tile([P, kb_last + 1, TLOC], BF16,
                                         tag="pT", bufs=6, name=f"{tag}_pT{h}")
        for p in (2 * g, 2 * g + 1):
            for kb, n0, mjs in plan:
                N = TLOC - n0 * P
                sps2 = []
                for half in range(2):
                    hb = 64 * half
                    sp = po["ps"].tile([P, 512], F32, tag="sps",
                                       name=f"{tag}_sp{p}_{kb}_{half}")
                    nc.tensor.matmul(
                        sp[:, :N],
                        kT[hb:hb + 64, p, kb * P:(kb + 1) * P],
                        qT[hb:hb + 64, p, n0 * P:],
                        start=True, stop=True, tile_position=(hb, 0))
                    sps2.append(sp)
                for half in range(2):
                    h = 2 * p + half
                    nc.scalar.activation(pTs[h][:, kb, n0 * P:],
                                         sps2[half][:, :N], AF.Exp)
                    for j in mjs:
                        nc.vector.tensor_tensor(
                            pTs[h][:, kb, j * P:(j + 1) * P],
                            pTs[h][:, kb, j * P:(j + 1) * P],
                            mask_sb[:, mask_idx[(kb, j)], :], ALU.mult)

    def emit_av_denom(g):
        for p in (2 * g, 2 * g + 1):
            zts[p] = po["psB"].tile([P, 512], F32, tag="zt",
                                    name=f"{tag}_zt{p}")
        dps[g] = po["psC"].tile([P, 512], F32, tag="dps", name=f"{tag}_dps{g}")
        nc.vector.memset(dps[g][0:97, :], 1.0)
        for kb, n0, _ in plan:
            for p in (2 * g, 2 * g + 1):
                for half in range(2):
                    h = 2 * p + half
                    hb = 64 * half
                    nc.tensor.matmul(
                        zts[p][hb:hb + 64, n0 * P:TLOC],
                        vR[:, kb, h * 64:(h + 1) * 64],
                        pTs[h][:, kb, n0 * P:],
                        start=(kb == kb_first), stop=(kb == kb_last),
                        tile_position=(0, hb))
            for gh in range(4):
                h = 4 * g + gh
                nc.tensor.matmul(
                    dps[g][32 * gh:32 * gh + 1, n0 * P:TLOC],
                    ones_bf[:, 0:1], pTs[h][:, kb, n0 * P:],
                    start=(kb == kb_first), stop=(kb == kb_last),
                    tile_position=(0, 32 * gh))

    def emit_norm(g):
        rsb = po["scr"].tile([P, TLOC], F32, tag="rsb", name=f"{tag}_rsb{g}")
        nc.vector.reciprocal_approx_fast(out=rsb[0:97, :],
                                         in_=dps[g][0:97, :TLOC])
        rsbb = po["scr"].tile([P, TLOC], BF16, tag="rsbb", name=f"{tag}_rb{g}")
        nc.vector.tensor_copy(rsbb[0:97, :], rsb[0:97, :])
        for p in (2 * g, 2 * g + 1):
            bcp = po["ps"].tile([P, 512], F32, tag="sps", name=f"{tag}_bc{p}")
            for half in range(2):
                h = 2 * p + half
                gh = h % 4
                hb = 64 * half
                nc.tensor.matmul(
                    bcp[hb:hb + 64, :TLOC],
                    ones_bf[32 * gh:32 * gh + 1, 0:64],
                    rsbb[32 * gh:32 * gh + 1, :TLOC],
                    start=True, stop=True, tile_position=(32 * gh, hb))
            bcs = po["scr"].tile([P, TLOC], F32, tag="bcs", name=f"{tag}_bs{p}")
            nc.vector.tensor_copy(bcs[:], bcp[:, :TLOC])
            nc.vector.tensor_tensor(oT[:, p, :], zts[p][:, :TLOC], bcs[:],
                                    ALU.mult)

    def emit_proj_group(g):
        # partial projection for this group's two head-pairs, accumulated
        # into out_pre via DVE adds (keeps PE fed inside the exp-bound
        # group window without holding 8 PSUM banks)
        for tb in range(NTB):
            prs = [po["ps"].tile([P, 512], F32, tag="sps",
                                 name=f"{tag}_pg{g}_{tb}_{eh}")
                   for eh in range(2)]
            for pi, p in enumerate((2 * g, 2 * g + 1)):
                for eh in range(2):
                    last = (pi == 1 and not (g == 3 and bias_sb is not None))
                    nc.tensor.matmul(prs[eh][:],
                                     oT[:, p, tb * P:(tb + 1) * P],
                                     wp_sb[:, p, eh * 512:(eh + 1) * 512],
                                     start=(pi == 0), stop=last)
            for eh in range(2):
                if g == 3 and bias_sb is not None:
                    nc.tensor.matmul(prs[eh][:], ones_bf[0:1, 0:P],
                                     bias_sb[0:1, eh * 512:(eh + 1) * 512],
                                     start=False, stop=True)
                src_in = resid if g == 0 else out_pre
                nc.vector.tensor_tensor(
                    out_pre[:, tb, eh * 512:(eh + 1) * 512], prs[eh][:],
                    src_in[:, tb, eh * 512:(eh + 1) * 512], ALU.add)
            if g == 3 and tb_cb is not None and tb > 0:
                tb_cb(tb - 1)
        if g == 3 and tb_cb is not None:
            tb_cb(NTB - 1)

    # pipelined group loop with filler PE work interleaved
    fillers = list(fillers)
    nfill = len(fillers)
    for g in range(4):
        if g > 0:
            emit_av_denom(g - 1)
        emit_scores(g)
        # spread filler units across the ACT-bound group windows
        lo = (g * nfill) // 4
        hi = ((g + 1) * nfill) // 4
        for f in fillers[lo:hi]:
            f()
        if g > 0:
            emit_norm(g - 1)
            if proj_by_group:
                emit_proj_group(g - 1)
    emit_av_denom(3)
    emit_norm(3)
    if proj_by_group:
        emit_proj_group(3)
        return

    # ---- projection + residual (+ optional bias via ones-matmul) ----
    for tb in range(NTB):
        prs = [po["ps"].tile([P, 512], F32, tag="sps",
                             name=f"{tag}_pr{tb}_{eh}") for eh in range(2)]
        for p in range(8):
            for eh in range(2):
                nc.tensor.matmul(prs[eh][:], oT[:, p, tb * P:(tb + 1) * P],
                                 wp_sb[:, p, eh * 512:(eh + 1) * 512],
                                 start=(p == 0),
                                 stop=(p == 7 and bias_sb is None))
        for eh in range(2):
            if bias_sb is not None:
                nc.tensor.matmul(prs[eh][:], ones_bf[0:1, 0:P],
                                 bias_sb[0:1, eh * 512:(eh + 1) * 512],
                                 start=False, stop=True)
            nc.vector.tensor_tensor(
                out_pre[:, tb, eh * 512:(eh + 1) * 512], prs[eh][:],
                resid[:, tb, eh * 512:(eh + 1) * 512], ALU.add)
        # lag the callback one tb so its DVE/ACT chain hides under the next
        # projection block's matmuls instead of stalling the PE
        if tb_cb is not None and tb > 0:
            tb_cb(tb - 1)
    if tb_cb is not None:
        tb_cb(NTB - 1)


def _ln_tb(nc, po, *, src, outR, tb, dst_T=None, gb=None, tag=""):
    """Layernorm of one 128-row block; optional bf16 transpose into dst_T.
    rstd computed as exp(-0.5*ln(var+eps)) to stay on one ACT table set."""
    stats = po["scr"].tile([P, 2, 6], F32, tag="ln_st", name=f"{tag}_st{tb}")
    nc.vector.bn_stats(stats[:, 0, :], src[:, tb, 0:512])
    nc.vector.bn_stats(stats[:, 1, :], src[:, tb, 512:1024])
    mv = po["scr"].tile([P, 2], F32, tag="ln_mv", name=f"{tag}_mv{tb}")
    nc.vector.bn_aggr(mv[:], stats[:])
    nmean = po["scr"].tile([P, 1], F32, tag="ln_nm", name=f"{tag}_nm{tb}")
    nc.vector.tensor_scalar_mul(nmean[:], mv[:, 0:1], -1.0)
    lnv = po["scr"].tile([P, 1], F32, tag="ln_lv", name=f"{tag}_lv{tb}")
    nc.scalar.activation(lnv[:], mv[:, 1:2], AF.Ln, bias=po["eps"][:])
    rstd = po["scr"].tile([P, 1], F32, tag="ln_rs", name=f"{tag}_rs{tb}")
    nc.scalar.activation(rstd[:], lnv[:], AF.Exp, scale=-0.5)
    nc.vector.tensor_scalar(outR[:, tb, :], src[:, tb, :],
                            nmean[:], rstd[:], ALU.add, ALU.mult)
    if gb is not None:
        g_bc, b_bc = gb
        nc.vector.tensor_tensor(outR[:, tb, :], outR[:, tb, :],
                                g_bc[:], ALU.mult)
        nc.vector.tensor_tensor(outR[:, tb, :], outR[:, tb, :],
                                b_bc[:], ALU.add)
    if dst_T is not None:
        ybf = po["scr"].tile([P, E], BF16, tag="ybf", name=f"{tag}_yb{tb}")
        nc.scalar.copy(ybf[:], outR[:, tb, :])
        for eg in range(2):
            pt = po["ps"].tile([P, 4, P], BF16, tag="sps",
                               name=f"{tag}_tr{tb}_{eg}")
            for j in range(4):
                ec = eg * 4 + j
                nc.tensor.transpose(pt[:, j, :], ybf[:, ec * P:(ec + 1) * P],
                                    po["ident"][:])
            for j in range(4):
                ec = eg * 4 + j
                nc.scalar.copy(dst_T[:, ec, tb * P:(tb + 1) * P],
                               pt[:, j, :])


def _broadcast_row(nc, po, src_row, width, tag):
    """Broadcast [1, width] f32 SBUF row (base 0) -> [128, width] f32."""
    out = po["persist"].tile([P, E], F32, tag=tag, name=tag)
    for c in range(0, width, 512):
        w = min(512, width - c)
        ps = po["ps"].tile([P, 512], F32, tag="sps", name=f"{tag}_bc{c}")
        nc.tensor.matmul(ps[0:P, :w], po["ones_f32c"][0:1, 0:P],
                         src_row[0:1, c:c + w], start=True, stop=True)
        nc.scalar.copy(out[:, c:c + w], ps[:, :w])
    return out


def build_program(sa_plan, ca_plan, use_gb):
    nc = bacc.Bacc("TRN2", target_bir_lowering=False, debug=False,
                   num_devices=8)
    kbs_s = max(e[0] for e in sa_plan) + 1
    kbs_c = max(e[0] for e in ca_plan) + 1
    SKV_S = kbs_s * P
    SKV_C = kbs_c * P
    nmask_s = sum(len(e[2]) for e in sa_plan)
    nmask_c = sum(len(e[2]) for e in ca_plan)

    def di(name, shape, dt=BF16):
        return nc.dram_tensor(name, shape, dt, kind="ExternalInput")

    xq = di("xq", [P, NEC, TLOC])
    xkv = di("xkv", [P, NEC, SKV_S])
    yencT = di("yencT", [P, NEC, SKV_C])
    xres = di("xres", [TLOC, E], F32)
    wq1 = di("wq1", [8, P, NEC, P]); wk1 = di("wk1", [8, P, NEC, P])
    wv1 = di("wv1", [P, NEC, E]); wp1 = di("wp1", [P, NEC, E])
    wq2 = di("wq2", [8, P, NEC, P]); wk2 = di("wk2", [8, P, NEC, P])
    wv2 = di("wv2", [P, NEC, E]); wp2 = di("wp2", [P, NEC, E])
    bp2 = di("bp2", [1, E])
    w1 = di("w1", [NFC, P, NEC, P]); b1c = di("b1c", [P, NFC], F32)
    w2 = di("w2", [P, NFC, E]); b2 = di("b2", [1, E])
    if nmask_s:
        masks_s = di("masks_s", [P, nmask_s, P])
    if nmask_c:
        masks_c = di("masks_c", [P, nmask_c, P])
    if use_gb:
        lngb = di("lngb", [1, 6 * E], F32)
    out = nc.dram_tensor("out", [TLOC, E], F32, kind="ExternalOutput")

    with tile.TileContext(nc) as tc:
        with (
            tc.tile_pool(name="persist", bufs=1) as persist,
            tc.tile_pool(name="rows", bufs=1) as rows,
            tc.tile_pool(name="cols", bufs=3) as cols,
            tc.tile_pool(name="wpool", bufs=2) as wpool,
            tc.tile_pool(name="wbig", bufs=2) as wbig,
            tc.tile_pool(name="scr", bufs=2) as scr,
        ):
            po = dict(persist=persist, rows=rows, cols=cols, w=wpool,
                      wbig=wbig, scr=scr)

            ones_bf = persist.tile([P, P], BF16)
            nc.vector.memset(ones_bf[:], 1.0)
            ident = persist.tile([P, P], BF16)
            make_identity(nc, ident[:])
            eps_tile = persist.tile([P, 1], F32)
            nc.vector.memset(eps_tile[:], 1e-5)
            po.update(ones_bf=ones_bf, ident=ident, eps=eps_tile)

            bp2_sb = persist.tile([1, E], BF16, tag="bp2", name="bp2s")
            nc.scalar.dma_start(bp2_sb[:], bp2[:])
            b2_sb = persist.tile([1, E], BF16, tag="b2", name="b2s")
            nc.scalar.dma_start(b2_sb[:], b2[:])
            b1_sb = persist.tile([P, NFC], F32, tag="b1c", name="b1s")
            nc.scalar.dma_start(b1_sb[:], b1c[:])

            rA = rows.tile([P, NTB, E], F32, tag="rA", name="rA")
            nc.scalar.dma_start(rA[:],
                                xres.rearrange("(tb p) e -> p tb e", p=P))

            gbs = [None, None, None]

            with (
                tc.tile_pool(name="attn_sb", bufs=2) as attn_sb,
                tc.tile_pool(name="kvpool", bufs=1) as kvpool,
                tc.tile_pool(name="psA", bufs=4, space="PSUM") as psA,
                tc.tile_pool(name="psB", bufs=2, space="PSUM") as psB,
                tc.tile_pool(name="psC", bufs=2, space="PSUM") as psC,
            ):
                po["attn"] = attn_sb
                po["ps"] = psA
                po["psB"] = psB
                po["psC"] = psC

                if use_gb:
                    ones_f32c = persist.tile([P, 1], F32, tag="onesf",
                                             name="onesf")
                    nc.vector.memset(ones_f32c[:], 1.0)
                    po.update(ones_f32c=ones_f32c)
                    gbrow = persist.tile([1, 6 * E], F32, tag="lngb",
                                         name="gbr")
                    nc.scalar.dma_start(gbrow[:], lngb[:])
                    for i in range(3):
                        g_bc = _broadcast_row(
                            nc, po, gbrow[:, 2 * i * E:(2 * i + 1) * E], E,
                            f"g_bc{i}")
                        b_bc = _broadcast_row(
                            nc, po, gbrow[:, (2 * i + 1) * E:(2 * i + 2) * E],
                            E, f"b_bc{i}")
                        gbs[i] = (g_bc, b_bc)

                xkv_sb = attn_sb.tile([P, NEC, SKV_S], BF16, tag="kvT",
                                      bufs=1, name="xkvS")
                nc.sync.dma_start(xkv_sb[:, 0:4, :], xkv[:, 0:4, :])
                nc.sync.dma_start(xkv_sb[:, 4:8, :], xkv[:, 4:8, :])
                xq_sb = cols.tile([P, NEC, TLOC], BF16, tag="colsBF",
                                  name="xqS")
                nc.sync.dma_start(xq_sb[:], xq[:])

                mask_s_sb = None
                if nmask_s:
                    mask_s_sb = attn_sb.tile([P, nmask_s, P], BF16,
                                             tag="mask_s", bufs=1,
                                             name="maskS")
                    nc.scalar.dma_start(mask_s_sb[:], masks_s[:])
                mask_c_sb = None
                if nmask_c:
                    mask_c_sb = attn_sb.tile([P, nmask_c, P], BF16,
                                             tag="mask_c", bufs=1,
                                             name="maskC")
                    nc.scalar.dma_start(mask_c_sb[:], masks_c[:])

                kT1 = kvpool.tile([P, NEC, SKV_S], BF16, tag="kT", bufs=1,
                                  name="kT1")
                vR1 = kvpool.tile([P, kbs_s, E], BF16, tag="vR", bufs=2,
                                  name="vR1")
                sa_vpre, sa_vunits = _attn_v_units(
                    nc, po, kvT=xkv_sb, plan=sa_plan, wvc=wv1, vR=vR1,
                    tag="sa")
                sa_vpre()
                _attn_k(nc, po, kvT=xkv_sb, plan=sa_plan, wkc=wk1, kT=kT1,
                        tag="sa")
                for u in sa_vunits:
                    u()

                # CA V-phase rides inside SA's exp-bound group windows as
                # PE filler; CA K covers LN1 afterwards.
                yencT_sb = attn_sb.tile([P, NEC, SKV_C], BF16, tag="kvT",
                                        bufs=1, name="yencS")
                nc.sync.dma_start(yencT_sb[:], yencT[:])
                kT2 = kvpool.tile([P, NEC, SKV_C], BF16, tag="kT", bufs=1,
                                  name="kT2")
                vR2 = kvpool.tile([P, kbs_c, E], BF16, tag="vR", bufs=2,
                                  name="vR2")
                ca_vpre, ca_vunits = _attn_v_units(
                    nc, po, kvT=yencT_sb, plan=ca_plan, wvc=wv2, vR=vR2,
                    tag="ca")
                ca_vpre()

                _attn_main(nc, po, yqT=xq_sb, plan=sa_plan,
                           mask_sb=mask_s_sb, wqc=wq1, wpc=wp1, kT=kT1,
                           vR=vR1, bias_sb=None, resid=rA, out_pre=rA,
                           tag="sa", fillers=ca_vunits)

                _attn_k(nc, po, kvT=yencT_sb, plan=ca_plan, wkc=wk2, kT=kT2,
                        tag="ca")

                rB = rows.tile([P, NTB, E], F32, tag="rB", name="rB")
                y1T = cols.tile([P, NEC, TLOC], BF16, tag="colsBF",
                                name="y1T")
                for tb in range(NTB):
                    _ln_tb(nc, po, src=rA, outR=rB, tb=tb, dst_T=y1T,
                           gb=gbs[0], tag="ln1")

                rC = rows.tile([P, NTB, E], F32, tag="rA", name="rC")
                y2T = cols.tile([P, NEC, TLOC], BF16, tag="colsBF",
                                name="y2T")

                def ln2_cb(tb):
                    _ln_tb(nc, po, src=rB, outR=rC, tb=tb, dst_T=y2T,
                           gb=gbs[1], tag="ln2")

                _attn_main(nc, po, yqT=y1T, plan=ca_plan, mask_sb=mask_c_sb,
                           wqc=wq2, wpc=wp2, kT=kT2, vR=vR2, bias_sb=bp2_sb,
                           resid=rB, out_pre=rB, tag="ca", tb_cb=ln2_cb)

            with tc.tile_pool(name="ffn_sb", bufs=1) as ffn_sb:
                hT = ffn_sb.tile([P, NFC, TLOC], BF16, tag="hT", name="hT")
                with tc.tile_pool(name="psF", bufs=4, space="PSUM") as psF:
                    po["ps"] = psF
                    for fc in range(NFC):
                        w1_t = wpool.tile([P, NEC, P], BF16, tag="w_qk",
                                          bufs=5, name=f"w1_{fc}")
                        nc.sync.dma_start(w1_t[:], w1[fc])
                        ps = psF.tile([P, 512], F32, tag="sps",
                                      name=f"psf1_{fc}")
                        for ec in range(NEC):
                            nc.tensor.matmul(ps[:, :TLOC], w1_t[:, ec, :],
                                             y2T[:, ec, :],
                                             start=(ec == 0),
                                             stop=(ec == NEC - 1))
                        nc.scalar.activation(hT[:, fc, :], ps[:, :TLOC],
                                             AF.Relu,
                                             bias=b1_sb[:, fc:fc + 1])

                with (
                    tc.tile_pool(name="psW2", bufs=8, space="PSUM") as psW2,
                    tc.tile_pool(name="w2pool", bufs=4) as w2pool,
                ):
                    pss = {}
                    for tb in range(NTB):
                        for eh in range(2):
                            pss[(tb, eh)] = psW2.tile(
                                [P, 512], F32, tag="f2", bufs=8,
                                name=f"psf2_{tb}_{eh}")
                    for fc in range(NFC):
                        w2_t = w2pool.tile([P, E], BF16, tag="w_f2", bufs=6,
                                           name=f"w2_{fc}")
                        nc.sync.dma_start(w2_t[:], w2[:, fc, :])
                        for tb in range(NTB):
                            for eh in range(2):
                                nc.tensor.matmul(
                                    pss[(tb, eh)][:],
                                    hT[:, fc, tb * P:(tb + 1) * P],
                                    w2_t[:, eh * 512:(eh + 1) * 512],
                                    start=(fc == 0), stop=False)
                    for tb in range(NTB):
                        for eh in range(2):
                            nc.tensor.matmul(
                                pss[(tb, eh)][:], po["ones_bf"][0:1, 0:P],
                                b2_sb[0:1, eh * 512:(eh + 1) * 512],
                                start=False, stop=True)
                            nc.vector.tensor_tensor(
                                rC[:, tb, eh * 512:(eh + 1) * 512],
                                pss[(tb, eh)][:],
                                rC[:, tb, eh * 512:(eh + 1) * 512], ALU.add)
                        _ln_tb(nc, po, src=rC, outR=rC, tb=tb, gb=gbs[2],
                               tag="ln3")
                        nc.sync.dma_start(out[tb * P:(tb + 1) * P, :],
                                          rC[:, tb, :])

    nc.compile()
    return nc


def _classify_blocks(masked):
    """masked: [ncore, Tq_local, Sk] bool (True = masked), core-row-gathered.

    Returns (plan, masks): plan = ((kb, n0, (masked_j,...)), ...) ascending kb
    over live key blocks; masks = per-core [128, nmask, 128] f32 keep masks
    (transposed to [s, t]) or None."""
    ncore = masked.shape[0]
    njb = masked.shape[1] // P
    nkb = masked.shape[2] // P
    state = np.zeros((njb, nkb), np.int32)  # 0 skip, 1 full, 2 masked
    for j in range(njb):
        for kb in range(nkb):
            blks = masked[:, j * P:(j + 1) * P, kb * P:(kb + 1) * P]
            if blks.all():
                state[j, kb] = 0
            elif not blks.any():
                state[j, kb] = 1
            else:
                state[j, kb] = 2
    plan = []
    for kb in range(nkb):
        lives = np.nonzero(state[:, kb])[0]
        if len(lives) == 0:
            continue
        n0 = int(lives[0])
        mjs = tuple(j for j in range(n0, njb) if state[j, kb] != 1)
        plan.append((kb, n0, mjs))
    ok = bool(plan) and plan[0][0] == 0 and plan[0][1] == 0
    if not ok:
        live_kb = [kb for kb in range(nkb)
                   if not masked[:, :, kb * P:(kb + 1) * P].all()]
        top = max(live_kb) + 1 if live_kb else 1
        plan = [(kb, 0, tuple(range(njb))) for kb in range(top)]
    masks = []
    for c in range(ncore):
        cols_ = []
        for kb, n0, mjs in plan:
            for j in mjs:
                keep = ~masked[c, j * P:(j + 1) * P, kb * P:(kb + 1) * P]
                cols_.append(np.ascontiguousarray(keep.T.astype(np.float32)))
        masks.append(np.stack(cols_, axis=1) if cols_ else None)
    return tuple(plan), masks


def _prep_inputs(inputs):
    """Host-side prep: returns (program_key, 8 in_maps, host_ln3)."""
    tgt = np.asarray(inputs["tgt"], np.float32)
    yenc = np.asarray(inputs["Y_enc_out"], np.float32)
    tgt_mask = np.asarray(inputs["tgt_mask"], np.float32)
    spm = np.asarray(inputs["src_padding_mask"])
    tpm = np.asarray(inputs["tgt_padding_mask"])

    causal = np.isneginf(tgt_mask) | np.isnan(tgt_mask)   # [Tq, Sk]
    masked_s = causal[None, :, :] | tpm[:, None, :]       # [B, Tq, Sk]
    masked_c = np.broadcast_to(spm[:, None, :], (B, T, T))

    def core_rows(th):
        return np.concatenate(
            [np.arange((th + 2 * j) * P, (th + 2 * j + 1) * P)
             for j in range(NTB)])

    rows_th = [core_rows(0), core_rows(1)]
    ms = np.stack([masked_s[b][rows_th[th]]
                   for b in range(B) for th in range(2)])
    mc = np.stack([masked_c[b][rows_th[th]]
                   for b in range(B) for th in range(2)])
    sa_plan, sa_masks = _classify_blocks(ms)
    ca_plan, ca_masks = _classify_blocks(mc)
    kbs_s = max(e[0] for e in sa_plan) + 1
    kbs_c = max(e[0] for e in ca_plan) + 1

    g1 = np.asarray(inputs["ln1_g"], np.float32)
    b1g = np.asarray(inputs["ln1_b"], np.float32)
    g2 = np.asarray(inputs["ln2_g"], np.float32)
    b2g = np.asarray(inputs["ln2_b"], np.float32)
    g3 = np.asarray(inputs["ln3_g"], np.float32)
    b3g = np.asarray(inputs["ln3_b"], np.float32)
    use_gb = not (np.all(g1 == 1) and np.all(g2 == 1) and np.all(b1g == 0)
                  and np.all(b2g == 0))
    host_ln3 = None
    if not (np.all(g3 == 1) and np.all(b3g == 0)):
        host_ln3 = (g3, b3g)

    def heads_cols(w):  # [H, E, DH] -> [E, H*DH]
        return np.ascontiguousarray(
            np.asarray(w, np.float32).transpose(1, 0, 2).reshape(E, E))

    def chunk_qk(w):  # [E, E] -> [8, 128, 8, 128] (p, pp, eo, c)
        return np.ascontiguousarray(
            w.reshape(8, P, 8, P).transpose(2, 1, 0, 3)).astype(BF16NP)

    def chunk_rows(w, n):  # [n*128, W] -> [128, n, W] (pp, chunk, c)
        return np.ascontiguousarray(
            w.reshape(n, P, -1).transpose(1, 0, 2)).astype(BF16NP)

    scale = 1.0 / np.sqrt(np.float32(DH))
    wq1 = chunk_qk(heads_cols(inputs["Wq1"]) * scale)
    wk1 = chunk_qk(heads_cols(inputs["Wk1"]))
    wv1 = chunk_rows(heads_cols(inputs["Wv1"]), NEC)
    wq2 = chunk_qk(heads_cols(inputs["Wq2"]) * scale)
    wk2 = chunk_qk(heads_cols(inputs["Wk2"]))
    wv2 = chunk_rows(heads_cols(inputs["Wv2"]), NEC)
    wp1 = chunk_rows(np.asarray(inputs["Wp1"], np.float32), NEC)
    wp2 = chunk_rows(np.asarray(inputs["Wp2"], np.float32), NEC)
    w1 = np.ascontiguousarray(
        np.asarray(inputs["W1"], np.float32).reshape(8, P, NFC, P)
        .transpose(2, 1, 0, 3)).astype(BF16NP)
    w2 = chunk_rows(np.asarray(inputs["W2"], np.float32), NFC)
    bp1 = np.asarray(inputs["bp1"], np.float32)
    bp2 = np.asarray(inputs["bp2"], np.float32).reshape(1, E).astype(BF16NP)
    b2v = np.asarray(inputs["b2"], np.float32).reshape(1, E).astype(BF16NP)
    b1c = np.ascontiguousarray(
        np.asarray(inputs["b1"], np.float32).reshape(NFC, P).T)
    lngb = np.concatenate([g1, b1g, g2, b2g, g3, b3g]).reshape(1, 6 * E)

    def tlayout(xT, ncols):  # [E, >=ncols] -> [128, 8, ncols]
        return np.ascontiguousarray(
            xT[:, :ncols].reshape(NEC, P, ncols).transpose(1, 0, 2)
        ).astype(BF16NP)

    in_maps = []
    for core in range(8):
        b = core // 2
        th = core % 2
        rows_ = rows_th[th]
        xb = tgt[:, b, :]
        xT = np.ascontiguousarray(xb.T)
        yT = np.ascontiguousarray(yenc[:, b, :].T)
        xqT = np.ascontiguousarray(xT[:, rows_])
        m = {
            "xq": np.ascontiguousarray(
                xqT.reshape(NEC, P, TLOC).transpose(1, 0, 2)).astype(BF16NP),
            "xkv": tlayout(xT, kbs_s * P),
            "yencT": tlayout(yT, kbs_c * P),
            "xres": np.ascontiguousarray(xb[rows_, :] + bp1[None, :]),
            "wq1": wq1, "wk1": wk1, "wv1": wv1, "wp1": wp1,
            "wq2": wq2, "wk2": wk2, "wv2": wv2, "wp2": wp2, "bp2": bp2,
            "w1": w1, "b1c": b1c, "w2": w2, "b2": b2v,
        }
        if sa_masks[core] is not None:
            m["masks_s"] = sa_masks[core].astype(BF16NP)
        if ca_masks[core] is not None:
            m["masks_c"] = ca_masks[core].astype(BF16NP)
        if use_gb:
            m["lngb"] = lngb
        in_maps.append(m)

    key = (sa_plan, ca_plan, use_gb)
    return key, in_maps, host_ln3


def kernel(**inputs) -> np.ndarray:
    key, in_maps, host_ln3 = _prep_inputs(inputs)
    if key not in _PROGRAM_CACHE:
        _PROGRAM_CACHE[key] = build_program(*key)
    nc = _PROGRAM_CACHE[key]
    res = run_bass_kernel_spmd(nc, in_maps, core_ids=list(range(8)))
    out = np.empty((T, B, E), np.float32)
    for core in range(8):
        b = core // 2
        th = core % 2
        for j in range(NTB):
            g = th + 2 * j
            out[g * P:(g + 1) * P, b, :] = \
                res.results[core]["out"][j * P:(j + 1) * P, :]
    if host_ln3 is not None:
        g3, b3g = host_ln3
        out = out * g3 + b3g
    return out
